# revision 31
# baseline (speedup 1.0000x reference)
"""Trainium2 Bass kernel for batched two-layer-MLP attention.

Reference semantics (per batch b):
    x  = sequence[:, b, :]                        # [S, D]
    K  = tanh(tanh(x @ Kw1.T) @ Kw2.T)
    Q  = tanh(tanh(x @ Qw1.T) @ Qw2.T)
    W  = softmax(K @ Q.T / sqrt(D), axis=-1)      # [S, S]
    out[:, b, :] = W @ x
Sharding: data-parallel over batch (B=8 -> 8 NeuronCores), weights replicated.

The scores and attended matmuls run in fp8(e4m3) with DoubleRow perf mode:
the PE packs 2 fp8 weights per cell, contracting 256 per instruction in the
same 512 cycles a bf16 matmul takes for 128 -- a full 2x (measured 216ns per
DR matmul at 2.4GHz, LDWEIGHTS hidden). Operand layouts keep k-pairs adjacent
so each DR matmul slices [128, 2, N] 3D APs out of the same tiles the bf16
version used. Accuracy (vs the 2e-2 gate): plain e4m3 on any single matmul
group costs 2.1-3.5e-2 end-to-end, so the MLP stays bf16 and the attended
matmul uses CENTERED weights: quantize exp(sc)-1 (logits are near-uniform,
so centering shrinks the fp8 quantization error ~2.5x) and add the rank-1
mean term back on the host. Measured end-to-end: 1.60e-2.

Layout strategy per core:
  - xt = x.T [D, S] bf16 host-pretransposed; weights pre-packed [p, j, k, c]
  - MLP outputs stay transposed: Kt, Qt [D, S] fp8
  - scores are computed TRANSPOSED: scT[t,s] = Q[t]-K[s] (lhsT = Qt 128-col
    slice, rhs = Kt chunk), so exp - WC lands directly in the attended
    matmul's lhsT layout as fp8 -- no xbar transpose, no fp8-convert chain
    (in the row-major formulation that chain's cross-queue WARs paced the PE)
  - softmax denominators (now partition-axis sums) via a tiny ones-matmul per
    s-block; 1/(rs + S*WC) folded into the PSUM->SBUF copy of the output
  - output written bf16; host adds the rank-1 mean term in fp32

Scheduling: HAM warmup matmuls fill the initial DMA wait; first-layer inputs
k/j-split across both HWDGE rings; phase B scores PSUM pool opened early.
"""

import numpy as np
import ml_dtypes

import concourse.bacc as bacc
import concourse.tile as tile
from concourse import mybir
from concourse.bass_utils import run_bass_kernel_spmd

P = 128          # partitions
S = 2048         # sequence length
D = 1024         # model dim
B = 8            # batch (one per core)
ST = S // P      # 16 s-tiles
DT = D // P      # 8 d-tiles
NF = 512         # psum free width (one bank of fp32)
SN = S // NF     # 4 score free-chunks
DN = D // NF     # 2 output free-chunks
BF = mybir.dt.bfloat16
F32 = mybir.dt.float32
E4 = mybir.dt.float8e4
SCALE = 1.0 / np.sqrt(np.float32(D))
WS = 32.0        # host-side weight pre-scale before fp8 cast

# per-stage precision switches (fp8 DoubleRow vs bf16)
MLP_FP8 = False
SC_FP8 = True
AT_FP8 = True
WC = 1.0         # attended-matmul W centering: quantize (exp(sc) - WC) in fp8;
                 # the rank-1 term WC * outer(1/rowsum, colsum(x)) is added
                 # back on the host (softmax logits are near-uniform, so
                 # centering shrinks fp8 quantization error ~2.5x)

DR = mybir.MatmulPerfMode.DoubleRow
AF = mybir.ActivationFunctionType

MLP_DT = E4 if MLP_FP8 else BF
SC_DT = E4 if SC_FP8 else BF
AT_DT = E4 if AT_FP8 else BF


def dr_matmuls(nc, ps, lhs_sl, rhs_sl, nk, fp8):
    """Accumulate nk 128-deep k-slices into ps; paired DoubleRow when fp8."""
    if fp8:
        for k2 in range(nk // 2):
            nc.tensor.matmul(ps, lhs_sl(2 * k2, 2), rhs_sl(2 * k2, 2),
                             start=(k2 == 0), stop=(k2 == nk // 2 - 1),
                             perf_mode=DR)
    else:
        for k in range(nk):
            nc.tensor.matmul(ps, lhs_sl(k, 1), rhs_sl(k, 1),
                             start=(k == 0), stop=(k == nk - 1))


def build_nc():
    nc = bacc.Bacc("TRN2", target_bir_lowering=False)

    xt_d = nc.dram_tensor("xt", [D, S], MLP_DT, kind="ExternalInput")
    xn_d = nc.dram_tensor("xn", [S, D], AT_DT, kind="ExternalInput")
    # head tensor: x.T's n=0 chunk pre-packed [p, kh, k%, s] so each k-half
    # loads with one fully-contiguous DMA (first matmul fires earliest)
    xh_d = nc.dram_tensor("xh", [P, 2, DT // 2, NF], MLP_DT, kind="ExternalInput")
    # weights pre-arranged on the host to [p, j, k, c] so each j-block loads
    # with one partition-contiguous DMA
    WSHAPE = [P, DT, DT, P]
    wk1_d = nc.dram_tensor("wk1", WSHAPE, MLP_DT, kind="ExternalInput")
    wk2_d = nc.dram_tensor("wk2", WSHAPE, MLP_DT, kind="ExternalInput")
    wq1_d = nc.dram_tensor("wq1", WSHAPE, MLP_DT, kind="ExternalInput")
    wq2_d = nc.dram_tensor("wq2", WSHAPE, MLP_DT, kind="ExternalInput")
    # bf16 output (halves the out-DMA traffic; the host fixup accumulates in
    # fp32 and bf16 adds only ~0.2% relative on top of the fp8 error)
    out_d = nc.dram_tensor("out", [S, D], BF, kind="ExternalOutput")
    # per-row 1/softmax-denominator, exported for the host rank-1 fixup
    rcp_d = nc.dram_tensor("rcpv", [S, 1], F32, kind="ExternalOutput")

    from contextlib import ExitStack

    with tile.TileContext(nc) as tc, ExitStack() as ctx:
        # ---- persistent SBUF arrays (live across both phases) ----
        pers = ctx.enter_context(tc.tile_pool(name="pers", bufs=1))
        xn_sb = pers.tile([P, ST, D], AT_DT)  # x normal: [t-part, t-tile, d]
        kt_sb = pers.tile([P, DT, S], SC_DT)  # K.T: [d-part, d-tile, s]
        # Q.T split per n-chunk so phase B's first scores don't wait on the
        # whole tensor's last tanh
        qt_n = [pers.tile([P, DT, NF], SC_DT, tag=f"qt{n}", name=f"qt{n}")
                for n in range(SN)]

        # scores PSUM pool opened before phase A so it gets banks disjoint
        # from the MLP pool - phase B's first matmul then has no released-pool
        # overlap dependency on phase A's tail
        psc = ctx.enter_context(tc.tile_pool(name="psum_sc", bufs=3, space="PSUM"))

        # ---- phase A: the four MLP layers ----
        with tc.tile_pool(name="phase_a", bufs=1) as pa, \
             tc.tile_pool(name="psum_mlp", bufs=4, space="PSUM") as pm:
            wp = pa  # weight tiles share the pool (fewer close barriers)
            # x.T split into per-n-chunk tiles so the first psum row's matmuls
            # only wait on the 1MB slice they read, not the whole array;
            # the n=0 chunk is additionally k-halved for an even earlier start
            KH = DT // 2
            xt_f = [pa.tile([P, KH, NF], MLP_DT, tag=f"xtf{h}", name=f"xtf{h}")
                    for h in range(2)]
            xt_n = [pa.tile([P, DT, NF], MLP_DT, tag=f"xt{n}", name=f"xt{n}")
                    for n in range(1, SN)]

            def xt_slice(n, k, w):
                if n == 0:
                    return xt_f[k // KH][:, k % KH:k % KH + w, :]
                return xt_n[n - 1][:, k:k + w, :]

            h1_sb = pa.tile([P, DT, S], MLP_DT)  # hidden activations (K then Q)

            # HAM warmup: throwaway matmuls while the first input DMAs are in
            # flight, so the real matmuls start at 2.4GHz
            warm_sb = pa.tile([P, NF], BF)
            nc.vector.memset(warm_sb, 0.0)
            warm_ps = pm.tile([P, NF], F32, tag="warm", bufs=1)
            NWARM = 13
            for i in range(NWARM):
                nc.tensor.matmul(warm_ps, warm_sb[:, 0:P], warm_sb,
                                 start=(i == 0), stop=(i == NWARM - 1))

            def mlp_layer(src, w_dram, dst, xdma=None, first=False):
                # dst[j, s] = tanh(scale * sum_k w[k, j].T @ src[k, s])
                # one tile + one DMA per j-block so dep granularity is per-j.
                xt_r = xt_d.rearrange("(k p) s -> p k s", p=P)
                if first:
                    w_jf = [wp.tile([P, KH, P], MLP_DT, tag=f"wf{h}",
                                    name=f"wf{h}", bufs=2) for h in range(2)]
                    w_j = [wp.tile([P, DT, P], MLP_DT, tag=f"w{j}",
                                   name=f"w{j}", bufs=2) for j in range(1, DT)]
                    for h in range(2):
                        nc.sync.dma_start(out=xt_f[h], in_=xh_d[:, h, :, :])
                        nc.scalar.dma_start(
                            out=w_jf[h], in_=w_dram[:, 0, h * KH:(h + 1) * KH, :])
                    for j in range(1, DT):
                        nc.scalar.dma_start(out=w_j[j - 1], in_=w_dram[:, j, :, :])
                    for n in range(1, SN):
                        nc.sync.dma_start(
                            out=xt_n[n - 1], in_=xt_r[:, :, n * NF:(n + 1) * NF])

                    def lhs_sl(j, k, w):
                        return (w_jf[k // KH][:, k % KH:k % KH + w, :] if j == 0
                                else w_j[j - 1][:, k:k + w, :])
                else:
                    w_j = [wp.tile([P, DT, P], MLP_DT, tag=f"w{j}",
                                   name=f"w{j}", bufs=2) for j in range(DT)]
                    for j in range(DT):
                        nc.sync.dma_start(out=w_j[j], in_=w_dram[:, j, :, :])
                    if xdma is not None:
                        xdma()

                    def lhs_sl(j, k, w):
                        return w_j[j][:, k:k + w, :]

                def rhs_sl(n, k, w):
                    return (xt_slice(n, k, w) if src is None
                            else src[:, k:k + w, n * NF:(n + 1) * NF])

                loop = ([(j, n) for n in range(SN) for j in range(DT)] if first
                        else [(j, n) for j in range(DT) for n in range(SN)])
                for j, n in loop:
                    ps = pm.tile([P, NF], F32, tag="mlp")
                    dr_matmuls(nc, ps,
                               lambda k, w: lhs_sl(j, k, w),
                               lambda k, w: rhs_sl(n, k, w),
                               DT, MLP_FP8)
                    dslice = (dst[n][:, j, :] if isinstance(dst, list)
                              else dst[:, j, n * NF:(n + 1) * NF])
                    nc.scalar.activation(out=dslice, in_=ps, func=AF.Tanh,
                                         scale=(1.0 / WS) if MLP_FP8 else 1.0)

            def load_xn():
                xn_r = xn_d.rearrange("(t p) d -> p t d", p=P)
                for t in range(0, ST, 4):
                    nc.sync.dma_start(out=xn_sb[:, t:t + 4, :],
                                      in_=xn_r[:, t:t + 4, :])

            mlp_layer(None, wk1_d, h1_sb, first=True)
            mlp_layer(h1_sb, wk2_d, kt_sb)
            mlp_layer(None, wq1_d, h1_sb, xdma=load_xn)
            mlp_layer(h1_sb, wq2_d, qt_n)

        # ---- phase B: transposed scores -> centered fp8 -> attended ----
        # scT[t, s] = Q[t].K[s] is computed with t on the psum partitions, so
        # exp lands DIRECTLY in the attended matmul's lhsT layout: no xbar
        # transpose, no cross-queue WAR coupling (the transpose chain was
        # pacing the PE in the row-major formulation). Softmax denominators
        # become partition-axis sums, recovered by a tiny ones-matmul per
        # s-block (~60 cycles/DR-pair on the PE).
        with tc.tile_pool(name="pb", bufs=1) as pb, \
             tc.tile_pool(name="psum_b", bufs=3, space="PSUM") as pbp:

            # Delta-W transposed, single-assignment: [t-part, t-block, s] fp8
            wq8 = pb.tile([P, ST, S], E4)
            ones8 = pb.tile([P, 2, 1], E4, tag="ones", name="ones8")
            nc.vector.memset(ones8, 1.0)

            # scores phase: exp(scT)-WC in fp8, s-chunk-major so the s-chunks
            # the first attended blocks need are finished long before the
            # phase boundary (no drain bubble)
            for n in range(SN):
                for tb in range(ST):
                    nq, qo = tb // SN, (tb % SN) * P
                    ps = psc.tile([P, NF], F32, tag="sc")
                    dr_matmuls(
                        nc, ps,
                        lambda k, w: qt_n[nq][:, k:k + w, qo:qo + P],
                        lambda k, w: kt_sb[:, k:k + w, n * NF:(n + 1) * NF],
                        DT, SC_FP8)
                    # scores are bounded (|sc/32| < ~3): exp without max-shift
                    est = pb.tile([P, NF], BF, tag="est", bufs=4)
                    nc.scalar.activation(out=est, in_=ps, func=AF.Exp,
                                         scale=float(SCALE))
                    nc.vector.tensor_scalar(
                        wq8[:, tb, n * NF:(n + 1) * NF], est, float(WC), None,
                        mybir.AluOpType.subtract)

            def attended(i):
                # softmax denominators for s-block i: rowsum of Delta-W via
                # ones-matmul (partition-axis sum), then 1/(rs + S*WC)
                rs = pbp.tile([P, 1], F32, tag="rs", bufs=2)
                for t2 in range(ST // 2):
                    nc.tensor.matmul(
                        rs, wq8[:, 2 * t2:2 * t2 + 2, i * P:(i + 1) * P],
                        ones8, start=(t2 == 0), stop=(t2 == ST // 2 - 1),
                        perf_mode=DR)
                rcp = pb.tile([P, 1], F32, tag="rcp", bufs=4)
                nc.vector.tensor_scalar(rcp, rs, float(S) * float(WC), None,
                                        mybir.AluOpType.add)
                nc.vector.reciprocal(rcp, rcp)
                nc.sync.dma_start(out=rcp_d[i * P:(i + 1) * P, :], in_=rcp)
                outst = pb.tile([P, D], BF, tag="outst", bufs=3)
                for n in range(DN):
                    ps = pbp.tile([P, NF], F32, tag="at")
                    dr_matmuls(
                        nc, ps,
                        lambda t, w: wq8[:, t:t + w, i * P:(i + 1) * P],
                        lambda t, w: xn_sb[:, t:t + w, n * NF:(n + 1) * NF],
                        ST, AT_FP8)
                    # fold the softmax normalization into the PSUM->SBUF copy
                    nc.scalar.mul(outst[:, n * NF:(n + 1) * NF], ps, rcp)
                    nc.sync.dma_start(
                        out=out_d[i * P:(i + 1) * P, n * NF:(n + 1) * NF],
                        in_=outst[:, n * NF:(n + 1) * NF],
                    )

            for i in range(ST):
                attended(i)

    nc.compile()
    return nc


_NC = None


def _get_nc():
    global _NC
    if _NC is None:
        _NC = build_nc()
    return _NC


NP_MLP = ml_dtypes.float8_e4m3 if MLP_FP8 else ml_dtypes.bfloat16
NP_SC = ml_dtypes.float8_e4m3 if SC_FP8 else ml_dtypes.bfloat16
NP_AT = ml_dtypes.float8_e4m3 if AT_FP8 else ml_dtypes.bfloat16


def _prep_w(w):
    """[d_out, d_in] f32 -> [p, j, k, c] of (WS*w.T) (k,p index d_in; j,c d_out)."""
    wt = np.asarray(w, dtype=np.float32).T
    if MLP_FP8:
        wt = wt * np.float32(WS)
    wt = wt.reshape(DT, P, DT, P).transpose(1, 2, 0, 3)
    return np.ascontiguousarray(wt).astype(NP_MLP)


def make_in_maps(sequence, Kw1, Kw2, Qw1, Qw2):
    seq = np.ascontiguousarray(np.transpose(np.asarray(sequence), (1, 0, 2)))  # [B, S, D]
    ws = {"wk1": _prep_w(Kw1), "wk2": _prep_w(Kw2),
          "wq1": _prep_w(Qw1), "wq2": _prep_w(Qw2)}
    in_maps = []
    colsums = []
    for b in range(B):
        xb = seq[b]
        xt = np.ascontiguousarray(xb.T).astype(NP_MLP)
        # [P, 2, KH, NF]: xh[p, h, q, s] = xt[(h*KH + q)*P + p, s] for s < NF
        xh = np.ascontiguousarray(
            xt[:, 0:NF].reshape(2, DT // 2, P, NF).transpose(2, 0, 1, 3))
        m = {"xn": xb.astype(NP_AT), "xt": xt, "xh": xh}
        m.update(ws)
        in_maps.append(m)
        colsums.append(xb.astype(np.float32).sum(axis=0))  # [D]
    return in_maps, colsums


def kernel(sequence, Kw1, Kw2, Qw1, Qw2):
    nc = _get_nc()
    in_maps, colsums = make_in_maps(sequence, Kw1, Kw2, Qw1, Qw2)
    res = run_bass_kernel_spmd(nc, in_maps, core_ids=list(range(B)))
    outs = []
    for b in range(B):
        ob = np.asarray(res.results[b]["out"], dtype=np.float32)
        if AT_FP8:
            # add back the rank-1 mean term removed by the W centering
            rcpv = np.asarray(res.results[b]["rcpv"], dtype=np.float32)[:, 0]
            ob = ob + np.float32(WC) * np.outer(rcpv, colsums[b])
        outs.append(ob)
    return np.stack(outs, axis=1).astype(np.float32)


# revision 40
# speedup vs baseline: 1.0330x; 1.0330x over previous
"""Trainium2 Bass kernel for batched two-layer-MLP attention.

Reference semantics (per batch b):
    x  = sequence[:, b, :]                        # [S, D]
    K  = tanh(tanh(x @ Kw1.T) @ Kw2.T)
    Q  = tanh(tanh(x @ Qw1.T) @ Qw2.T)
    W  = softmax(K @ Q.T / sqrt(D), axis=-1)      # [S, S]
    out[:, b, :] = W @ x
Sharding: data-parallel over batch (B=8 -> 8 NeuronCores), weights replicated.

The scores and attended matmuls run in fp8(e4m3) with DoubleRow perf mode:
the PE packs 2 fp8 weights per cell, contracting 256 per instruction in the
same 512 cycles a bf16 matmul takes for 128 -- a full 2x (measured 216ns per
DR matmul at 2.4GHz, LDWEIGHTS hidden). Operand layouts keep k-pairs adjacent
so each DR matmul slices [128, 2, N] 3D APs out of the same tiles the bf16
version used. Accuracy (vs the 2e-2 gate): plain e4m3 on any single matmul
group costs 2.1-3.5e-2 end-to-end, so the MLP stays bf16 and the attended
matmul uses CENTERED weights: quantize exp(sc)-1 (logits are near-uniform,
so centering shrinks the fp8 quantization error ~2.5x) and add the rank-1
mean term back on the host. Measured end-to-end: 1.60e-2.

Layout strategy per core:
  - xt = x.T [D, S] bf16 host-pretransposed; weights pre-packed [p, j, k, c]
  - MLP outputs stay transposed: Kt, Qt [D, S] fp8
  - scores are computed TRANSPOSED: scT[t,s] = Q[t]-K[s] (lhsT = Qt 128-col
    slice, rhs = Kt chunk), so exp - WC lands directly in the attended
    matmul's lhsT layout as fp8 -- no xbar transpose, no fp8-convert chain
    (in the row-major formulation that chain's cross-queue WARs paced the PE)
  - softmax denominators (now partition-axis sums) via a tiny ones-matmul per
    s-block; 1/(rs + S*WC) folded into the PSUM->SBUF copy of the output
  - output written bf16; host adds the rank-1 mean term in fp32

Scheduling: HAM warmup matmuls fill the initial DMA wait; first-layer inputs
k/j-split across both HWDGE rings; phase B scores PSUM pool opened early.
"""

import numpy as np
import ml_dtypes

import concourse.bacc as bacc
import concourse.tile as tile
from concourse import mybir
from concourse.bass_utils import run_bass_kernel_spmd

P = 128          # partitions
S = 2048         # sequence length
D = 1024         # model dim
B = 8            # batch (one per core)
ST = S // P      # 16 s-tiles
DT = D // P      # 8 d-tiles
NF = 512         # psum free width (one bank of fp32)
SN = S // NF     # 4 score free-chunks
DN = D // NF     # 2 output free-chunks
BF = mybir.dt.bfloat16
F32 = mybir.dt.float32
E4 = mybir.dt.float8e4
SCALE = 1.0 / np.sqrt(np.float32(D))
WS = 32.0        # host-side weight pre-scale before fp8 cast

# per-stage precision switches (fp8 DoubleRow vs bf16)
MLP_FP8 = False
MLP2_MIX = True  # k-slices 0,1 of both second MLP layers as one fp8 DR pair
                 # (25% of that contraction; costs ~1.1e-2 error in quadrature,
                 # saves ~14us of PE). Operands are cast to fp8 UNSCALED: e4m3
                 # subnormals carry tiny absolute error, and unscaled products
                 # accumulate into the same psum as the bf16 k-slices.
SC_FP8 = True
AT_FP8 = True
WC = 1.0         # attended-matmul W centering: quantize (exp(sc) - WC) in fp8;
                 # the rank-1 term WC * outer(1/rowsum, colsum(x)) is added
                 # back on the host (softmax logits are near-uniform, so
                 # centering shrinks fp8 quantization error ~2.5x)

DR = mybir.MatmulPerfMode.DoubleRow
AF = mybir.ActivationFunctionType

MLP_DT = E4 if MLP_FP8 else BF
SC_DT = E4 if SC_FP8 else BF
AT_DT = E4 if AT_FP8 else BF


def dr_matmuls(nc, ps, lhs_sl, rhs_sl, nk, fp8):
    """Accumulate nk 128-deep k-slices into ps; paired DoubleRow when fp8."""
    if fp8:
        for k2 in range(nk // 2):
            nc.tensor.matmul(ps, lhs_sl(2 * k2, 2), rhs_sl(2 * k2, 2),
                             start=(k2 == 0), stop=(k2 == nk // 2 - 1),
                             perf_mode=DR)
    else:
        for k in range(nk):
            nc.tensor.matmul(ps, lhs_sl(k, 1), rhs_sl(k, 1),
                             start=(k == 0), stop=(k == nk - 1))


def build_nc():
    nc = bacc.Bacc("TRN2", target_bir_lowering=False)

    xt_d = nc.dram_tensor("xt", [D, S], MLP_DT, kind="ExternalInput")
    xn_d = nc.dram_tensor("xn", [S, D], AT_DT, kind="ExternalInput")
    # head tensor: x.T's n=0 chunk pre-packed [p, kh, k%, s] so each k-half
    # loads with one fully-contiguous DMA (first matmul fires earliest)
    xh_d = nc.dram_tensor("xh", [P, 2, DT // 2, NF], MLP_DT, kind="ExternalInput")
    # weights pre-arranged on the host to [p, j, k, c] so each j-block loads
    # with one partition-contiguous DMA
    WSHAPE = [P, DT, DT, P]
    wk1_d = nc.dram_tensor("wk1", WSHAPE, MLP_DT, kind="ExternalInput")
    wk2_d = nc.dram_tensor("wk2", WSHAPE, MLP_DT, kind="ExternalInput")
    wq1_d = nc.dram_tensor("wq1", WSHAPE, MLP_DT, kind="ExternalInput")
    wq2_d = nc.dram_tensor("wq2", WSHAPE, MLP_DT, kind="ExternalInput")
    if MLP2_MIX:
        # fp8 copies of k-slices 0,1 of the second-layer weights (unscaled)
        W8SHAPE = [P, DT, 2, P]
        wk2p_d = nc.dram_tensor("wk2p", W8SHAPE, E4, kind="ExternalInput")
        wq2p_d = nc.dram_tensor("wq2p", W8SHAPE, E4, kind="ExternalInput")
    # bf16 output (halves the out-DMA traffic; the host fixup accumulates in
    # fp32 and bf16 adds only ~0.2% relative on top of the fp8 error)
    out_d = nc.dram_tensor("out", [S, D], BF, kind="ExternalOutput")
    # per-row 1/softmax-denominator, exported for the host rank-1 fixup
    rcp_d = nc.dram_tensor("rcpv", [S, 1], F32, kind="ExternalOutput")

    from contextlib import ExitStack

    with tile.TileContext(nc) as tc, ExitStack() as ctx:
        # ---- persistent SBUF arrays (live across both phases) ----
        pers = ctx.enter_context(tc.tile_pool(name="pers", bufs=1))
        xn_sb = pers.tile([P, ST, D], AT_DT)  # x normal: [t-part, t-tile, d]
        kt_sb = pers.tile([P, DT, S], SC_DT)  # K.T: [d-part, d-tile, s]
        # Q.T split per n-chunk so phase B's first scores don't wait on the
        # whole tensor's last tanh
        qt_n = [pers.tile([P, DT, NF], SC_DT, tag=f"qt{n}", name=f"qt{n}")
                for n in range(SN)]

        # scores PSUM pool opened before phase A so it gets banks disjoint
        # from the MLP pool - phase B's first matmul then has no released-pool
        # overlap dependency on phase A's tail
        psc = ctx.enter_context(tc.tile_pool(name="psum_sc", bufs=3, space="PSUM"))

        # ---- phase A: the four MLP layers ----
        with tc.tile_pool(name="phase_a", bufs=1) as pa, \
             tc.tile_pool(name="psum_mlp", bufs=4, space="PSUM") as pm:
            wp = pa  # weight tiles share the pool (fewer close barriers)
            # x.T split into per-n-chunk tiles so the first psum row's matmuls
            # only wait on the 1MB slice they read, not the whole array;
            # the n=0 chunk is additionally k-halved for an even earlier start
            KH = DT // 2
            xt_f = [pa.tile([P, KH, NF], MLP_DT, tag=f"xtf{h}", name=f"xtf{h}")
                    for h in range(2)]
            xt_n = [pa.tile([P, DT, NF], MLP_DT, tag=f"xt{n}", name=f"xt{n}")
                    for n in range(1, SN)]

            def xt_slice(n, k, w):
                if n == 0:
                    return xt_f[k // KH][:, k % KH:k % KH + w, :]
                return xt_n[n - 1][:, k:k + w, :]

            h1_sb = pa.tile([P, DT, S], MLP_DT)  # hidden activations (K then Q)
            # fp8 copy of hidden slices j=0,1 (the mixed DR pair's rhs);
            # layer 1's tanh writes these directly when MLP2_MIX
            h1_8 = pa.tile([P, 2, S], E4, name="h1_8") if MLP2_MIX else None

            # HAM warmup: throwaway matmuls while the first input DMAs are in
            # flight, so the real matmuls start at 2.4GHz
            warm_sb = pa.tile([P, NF], BF)
            nc.vector.memset(warm_sb, 0.0)
            warm_ps = pm.tile([P, NF], F32, tag="warm", bufs=1)
            NWARM = 13
            for i in range(NWARM):
                nc.tensor.matmul(warm_ps, warm_sb[:, 0:P], warm_sb,
                                 start=(i == 0), stop=(i == NWARM - 1))

            def mlp_layer(src, w_dram, dst, xdma=None, first=False,
                          w8_dram=None, dst8=None):
                # dst[j, s] = tanh(scale * sum_k w[k, j].T @ src[k, s])
                # one tile + one DMA per j-block so dep granularity is per-j.
                # w8_dram: fp8 weights for k-slices 0,1 -> one DR pair per
                # group (reads src's fp8 copy h1_8). dst8: fp8 destination
                # for output slices j=0,1 (feeds the next layer's DR pair).
                xt_r = xt_d.rearrange("(k p) s -> p k s", p=P)
                if first:
                    w_jf = [wp.tile([P, KH, P], MLP_DT, tag=f"wf{h}",
                                    name=f"wf{h}", bufs=2) for h in range(2)]
                    w_j = [wp.tile([P, DT, P], MLP_DT, tag=f"w{j}",
                                   name=f"w{j}", bufs=2) for j in range(1, DT)]
                    for h in range(2):
                        nc.sync.dma_start(out=xt_f[h], in_=xh_d[:, h, :, :])
                        nc.scalar.dma_start(
                            out=w_jf[h], in_=w_dram[:, 0, h * KH:(h + 1) * KH, :])
                    for j in range(1, DT):
                        nc.scalar.dma_start(out=w_j[j - 1], in_=w_dram[:, j, :, :])
                    for n in range(1, SN):
                        nc.sync.dma_start(
                            out=xt_n[n - 1], in_=xt_r[:, :, n * NF:(n + 1) * NF])

                    def lhs_sl(j, k, w):
                        return (w_jf[k // KH][:, k % KH:k % KH + w, :] if j == 0
                                else w_j[j - 1][:, k:k + w, :])
                else:
                    w_j = [wp.tile([P, DT, P], MLP_DT, tag=f"w{j}",
                                   name=f"w{j}", bufs=2) for j in range(DT)]
                    for j in range(DT):
                        nc.sync.dma_start(out=w_j[j], in_=w_dram[:, j, :, :])
                    if w8_dram is not None:
                        w8_sb = wp.tile([P, DT, 2, P], E4, tag="w8",
                                        name="w8", bufs=2)
                        nc.scalar.dma_start(out=w8_sb, in_=w8_dram[:, :, :, :])
                    if xdma is not None:
                        xdma()

                    def lhs_sl(j, k, w):
                        return w_j[j][:, k:k + w, :]

                def rhs_sl(n, k, w):
                    return (xt_slice(n, k, w) if src is None
                            else src[:, k:k + w, n * NF:(n + 1) * NF])

                loop = ([(j, n) for n in range(SN) for j in range(DT)] if first
                        else [(j, n) for j in range(DT) for n in range(SN)])
                for j, n in loop:
                    ps = pm.tile([P, NF], F32, tag="mlp")
                    if w8_dram is not None:
                        # k=0,1 as one fp8 DR pair, k=2..7 bf16 into the
                        # same accumulation group
                        nc.tensor.matmul(
                            ps, w8_sb[:, j, :, :],
                            h1_8[:, :, n * NF:(n + 1) * NF],
                            start=True, stop=False, perf_mode=DR)
                        for k in range(2, DT):
                            nc.tensor.matmul(ps, lhs_sl(j, k, 1),
                                             rhs_sl(n, k, 1),
                                             start=False, stop=(k == DT - 1))
                    else:
                        dr_matmuls(nc, ps,
                                   lambda k, w: lhs_sl(j, k, w),
                                   lambda k, w: rhs_sl(n, k, w),
                                   DT, MLP_FP8)
                    if dst8 is not None and j < 2:
                        dslice = dst8[:, j, n * NF:(n + 1) * NF]
                    else:
                        dslice = (dst[n][:, j, :] if isinstance(dst, list)
                                  else dst[:, j, n * NF:(n + 1) * NF])
                    nc.scalar.activation(out=dslice, in_=ps, func=AF.Tanh,
                                         scale=(1.0 / WS) if MLP_FP8 else 1.0)

            def load_xn():
                xn_r = xn_d.rearrange("(t p) d -> p t d", p=P)
                for t in range(0, ST, 4):
                    nc.sync.dma_start(out=xn_sb[:, t:t + 4, :],
                                      in_=xn_r[:, t:t + 4, :])

            if MLP2_MIX:
                mlp_layer(None, wk1_d, h1_sb, first=True, dst8=h1_8)
                mlp_layer(h1_sb, wk2_d, kt_sb, w8_dram=wk2p_d)
                mlp_layer(None, wq1_d, h1_sb, xdma=load_xn, dst8=h1_8)
                mlp_layer(h1_sb, wq2_d, qt_n, w8_dram=wq2p_d)
            else:
                mlp_layer(None, wk1_d, h1_sb, first=True)
                mlp_layer(h1_sb, wk2_d, kt_sb)
                mlp_layer(None, wq1_d, h1_sb, xdma=load_xn)
                mlp_layer(h1_sb, wq2_d, qt_n)

        # ---- phase B: transposed scores -> centered fp8 -> attended ----
        # scT[t, s] = Q[t].K[s] is computed with t on the psum partitions, so
        # exp lands DIRECTLY in the attended matmul's lhsT layout: no xbar
        # transpose, no cross-queue WAR coupling (the transpose chain was
        # pacing the PE in the row-major formulation). Softmax denominators
        # become partition-axis sums, recovered by a tiny ones-matmul per
        # s-block (~60 cycles/DR-pair on the PE).
        with tc.tile_pool(name="pb", bufs=1) as pb, \
             tc.tile_pool(name="psum_b", bufs=3, space="PSUM") as pbp:

            # Delta-W transposed, single-assignment: [t-part, t-block, s] fp8
            wq8 = pb.tile([P, ST, S], E4)
            ones8 = pb.tile([P, 2, 1], E4, tag="ones", name="ones8")
            nc.vector.memset(ones8, 1.0)

            # scores phase: exp(scT)-WC in fp8, s-chunk-major so the s-chunks
            # the first attended blocks need are finished long before the
            # phase boundary (no drain bubble)
            for n in range(SN):
                for tb in range(ST):
                    nq, qo = tb // SN, (tb % SN) * P
                    ps = psc.tile([P, NF], F32, tag="sc")
                    dr_matmuls(
                        nc, ps,
                        lambda k, w: qt_n[nq][:, k:k + w, qo:qo + P],
                        lambda k, w: kt_sb[:, k:k + w, n * NF:(n + 1) * NF],
                        DT, SC_FP8)
                    # scores are bounded (|sc/32| < ~3): exp without max-shift
                    est = pb.tile([P, NF], BF, tag="est", bufs=4)
                    nc.scalar.activation(out=est, in_=ps, func=AF.Exp,
                                         scale=float(SCALE))
                    nc.vector.tensor_scalar(
                        wq8[:, tb, n * NF:(n + 1) * NF], est, float(WC), None,
                        mybir.AluOpType.subtract)

            def attended(i):
                # softmax denominators for s-block i: rowsum of Delta-W via
                # ones-matmul (partition-axis sum), then 1/(rs + S*WC)
                rs = pbp.tile([P, 1], F32, tag="rs", bufs=2)
                for t2 in range(ST // 2):
                    nc.tensor.matmul(
                        rs, wq8[:, 2 * t2:2 * t2 + 2, i * P:(i + 1) * P],
                        ones8, start=(t2 == 0), stop=(t2 == ST // 2 - 1),
                        perf_mode=DR)
                rcp = pb.tile([P, 1], F32, tag="rcp", bufs=4)
                nc.vector.tensor_scalar(rcp, rs, float(S) * float(WC), None,
                                        mybir.AluOpType.add)
                nc.vector.reciprocal(rcp, rcp)
                nc.sync.dma_start(out=rcp_d[i * P:(i + 1) * P, :], in_=rcp)
                outst = pb.tile([P, D], BF, tag="outst", bufs=3)
                for n in range(DN):
                    ps = pbp.tile([P, NF], F32, tag="at")
                    dr_matmuls(
                        nc, ps,
                        lambda t, w: wq8[:, t:t + w, i * P:(i + 1) * P],
                        lambda t, w: xn_sb[:, t:t + w, n * NF:(n + 1) * NF],
                        ST, AT_FP8)
                    # fold the softmax normalization into the PSUM->SBUF copy
                    nc.scalar.mul(outst[:, n * NF:(n + 1) * NF], ps, rcp)
                    nc.sync.dma_start(
                        out=out_d[i * P:(i + 1) * P, n * NF:(n + 1) * NF],
                        in_=outst[:, n * NF:(n + 1) * NF],
                    )

            for i in range(ST):
                attended(i)

    nc.compile()
    return nc


_NC = None


def _get_nc():
    global _NC
    if _NC is None:
        _NC = build_nc()
    return _NC


NP_MLP = ml_dtypes.float8_e4m3 if MLP_FP8 else ml_dtypes.bfloat16
NP_SC = ml_dtypes.float8_e4m3 if SC_FP8 else ml_dtypes.bfloat16
NP_AT = ml_dtypes.float8_e4m3 if AT_FP8 else ml_dtypes.bfloat16


def _prep_w(w):
    """[d_out, d_in] f32 -> [p, j, k, c] of (WS*w.T) (k,p index d_in; j,c d_out)."""
    wt = np.asarray(w, dtype=np.float32).T
    if MLP_FP8:
        wt = wt * np.float32(WS)
    wt = wt.reshape(DT, P, DT, P).transpose(1, 2, 0, 3)
    return np.ascontiguousarray(wt).astype(NP_MLP)


def _prep_w8(w):
    """k-slices 0,1 of w.T as [p, j, 2, c] e4m3, unscaled."""
    wt = np.asarray(w, dtype=np.float32).T
    wt = wt.reshape(DT, P, DT, P).transpose(1, 2, 0, 3)[:, :, 0:2, :]
    return np.ascontiguousarray(wt).astype(ml_dtypes.float8_e4m3)


def make_in_maps(sequence, Kw1, Kw2, Qw1, Qw2):
    seq = np.ascontiguousarray(np.transpose(np.asarray(sequence), (1, 0, 2)))  # [B, S, D]
    ws = {"wk1": _prep_w(Kw1), "wk2": _prep_w(Kw2),
          "wq1": _prep_w(Qw1), "wq2": _prep_w(Qw2)}
    if MLP2_MIX:
        ws["wk2p"] = _prep_w8(Kw2)
        ws["wq2p"] = _prep_w8(Qw2)
    in_maps = []
    colsums = []
    for b in range(B):
        xb = seq[b]
        xt = np.ascontiguousarray(xb.T).astype(NP_MLP)
        # [P, 2, KH, NF]: xh[p, h, q, s] = xt[(h*KH + q)*P + p, s] for s < NF
        xh = np.ascontiguousarray(
            xt[:, 0:NF].reshape(2, DT // 2, P, NF).transpose(2, 0, 1, 3))
        m = {"xn": xb.astype(NP_AT), "xt": xt, "xh": xh}
        m.update(ws)
        in_maps.append(m)
        colsums.append(xb.astype(np.float32).sum(axis=0))  # [D]
    return in_maps, colsums


def kernel(sequence, Kw1, Kw2, Qw1, Qw2):
    nc = _get_nc()
    in_maps, colsums = make_in_maps(sequence, Kw1, Kw2, Qw1, Qw2)
    res = run_bass_kernel_spmd(nc, in_maps, core_ids=list(range(B)))
    outs = []
    for b in range(B):
        ob = np.asarray(res.results[b]["out"], dtype=np.float32)
        if AT_FP8:
            # add back the rank-1 mean term removed by the W centering
            rcpv = np.asarray(res.results[b]["rcpv"], dtype=np.float32)[:, 0]
            ob = ob + np.float32(WC) * np.outer(rcpv, colsums[b])
        outs.append(ob)
    return np.stack(outs, axis=1).astype(np.float32)


# revision 42
# speedup vs baseline: 1.0454x; 1.0120x over previous
"""Trainium2 Bass kernel for batched two-layer-MLP attention.

Reference semantics (per batch b):
    x  = sequence[:, b, :]                        # [S, D]
    K  = tanh(tanh(x @ Kw1.T) @ Kw2.T)
    Q  = tanh(tanh(x @ Qw1.T) @ Qw2.T)
    W  = softmax(K @ Q.T / sqrt(D), axis=-1)      # [S, S]
    out[:, b, :] = W @ x
Sharding: data-parallel over batch (B=8 -> 8 NeuronCores), weights replicated.

The scores and attended matmuls run in fp8(e4m3) with DoubleRow perf mode:
the PE packs 2 fp8 weights per cell, contracting 256 per instruction in the
same 512 cycles a bf16 matmul takes for 128 -- a full 2x (measured 216ns per
DR matmul at 2.4GHz, LDWEIGHTS hidden). Operand layouts keep k-pairs adjacent
so each DR matmul slices [128, 2, N] 3D APs out of the same tiles the bf16
version used. Accuracy (vs the 2e-2 gate): plain e4m3 on any single matmul
group costs 2.1-3.5e-2 end-to-end, so the MLP stays bf16 and the attended
matmul uses CENTERED weights: quantize exp(sc)-1 (logits are near-uniform,
so centering shrinks the fp8 quantization error ~2.5x) and add the rank-1
mean term back on the host. Measured end-to-end: 1.60e-2.

Layout strategy per core:
  - xt = x.T [D, S] bf16 host-pretransposed; weights pre-packed [p, j, k, c]
  - MLP outputs stay transposed: Kt, Qt [D, S] fp8
  - scores are computed TRANSPOSED: scT[t,s] = Q[t]-K[s] (lhsT = Qt 128-col
    slice, rhs = Kt chunk), so exp - WC lands directly in the attended
    matmul's lhsT layout as fp8 -- no xbar transpose, no fp8-convert chain
    (in the row-major formulation that chain's cross-queue WARs paced the PE)
  - softmax denominators (now partition-axis sums) via a tiny ones-matmul per
    s-block; 1/(rs + S*WC) folded into the PSUM->SBUF copy of the output
  - output written bf16; host adds the rank-1 mean term in fp32

Scheduling: HAM warmup matmuls fill the initial DMA wait; first-layer inputs
k/j-split across both HWDGE rings; phase B scores PSUM pool opened early.
"""

import numpy as np
import ml_dtypes

import concourse.bacc as bacc
import concourse.tile as tile
from concourse import mybir
from concourse.bass_utils import run_bass_kernel_spmd

P = 128          # partitions
S = 2048         # sequence length
D = 1024         # model dim
B = 8            # batch (one per core)
ST = S // P      # 16 s-tiles
DT = D // P      # 8 d-tiles
NF = 512         # psum free width (one bank of fp32)
SN = S // NF     # 4 score free-chunks
DN = D // NF     # 2 output free-chunks
BF = mybir.dt.bfloat16
F32 = mybir.dt.float32
E4 = mybir.dt.float8e4
SCALE = 1.0 / np.sqrt(np.float32(D))
WS = 32.0        # host-side weight pre-scale before fp8 cast

# per-stage precision switches (fp8 DoubleRow vs bf16)
MLP_FP8 = False
MLP2_MIX = True  # k-slices 0,1 of both second MLP layers as one fp8 DR pair
                 # (25% of that contraction; costs ~1.1e-2 error in quadrature,
                 # saves ~14us of PE). Operands are cast to fp8 UNSCALED: e4m3
                 # subnormals carry tiny absolute error, and unscaled products
                 # accumulate into the same psum as the bf16 k-slices.
SC_FP8 = True
AT_FP8 = True
WC = 1.0         # attended-matmul W centering: quantize (exp(sc) - WC) in fp8;
                 # the rank-1 term WC * outer(1/rowsum, colsum(x)) is added
                 # back on the host (softmax logits are near-uniform, so
                 # centering shrinks fp8 quantization error ~2.5x)

DR = mybir.MatmulPerfMode.DoubleRow
AF = mybir.ActivationFunctionType

MLP_DT = E4 if MLP_FP8 else BF
SC_DT = E4 if SC_FP8 else BF
AT_DT = E4 if AT_FP8 else BF


def dr_matmuls(nc, ps, lhs_sl, rhs_sl, nk, fp8):
    """Accumulate nk 128-deep k-slices into ps; paired DoubleRow when fp8."""
    if fp8:
        for k2 in range(nk // 2):
            nc.tensor.matmul(ps, lhs_sl(2 * k2, 2), rhs_sl(2 * k2, 2),
                             start=(k2 == 0), stop=(k2 == nk // 2 - 1),
                             perf_mode=DR)
    else:
        for k in range(nk):
            nc.tensor.matmul(ps, lhs_sl(k, 1), rhs_sl(k, 1),
                             start=(k == 0), stop=(k == nk - 1))


def build_nc():
    nc = bacc.Bacc("TRN2", target_bir_lowering=False)

    xt_d = nc.dram_tensor("xt", [D, S], MLP_DT, kind="ExternalInput")
    xn_d = nc.dram_tensor("xn", [S, D], AT_DT, kind="ExternalInput")
    # head tensor: x.T's n=0 chunk pre-packed [p, kh, k%, s] so each k-half
    # loads with one fully-contiguous DMA (first matmul fires earliest)
    xh_d = nc.dram_tensor("xh", [P, 2, DT // 2, NF], MLP_DT, kind="ExternalInput")
    # weights pre-arranged on the host to [p, j, k, c] so each j-block loads
    # with one partition-contiguous DMA
    WSHAPE = [P, DT, DT, P]
    wk1_d = nc.dram_tensor("wk1", WSHAPE, MLP_DT, kind="ExternalInput")
    wk2_d = nc.dram_tensor("wk2", WSHAPE, MLP_DT, kind="ExternalInput")
    wq1_d = nc.dram_tensor("wq1", WSHAPE, MLP_DT, kind="ExternalInput")
    wq2_d = nc.dram_tensor("wq2", WSHAPE, MLP_DT, kind="ExternalInput")
    if MLP2_MIX:
        # fp8 copies of k-slices 0,1 of the second-layer weights (unscaled)
        W8SHAPE = [P, DT, 2, P]
        wk2p_d = nc.dram_tensor("wk2p", W8SHAPE, E4, kind="ExternalInput")
        wq2p_d = nc.dram_tensor("wq2p", W8SHAPE, E4, kind="ExternalInput")
    # bf16 output (halves the out-DMA traffic; the host fixup accumulates in
    # fp32 and bf16 adds only ~0.2% relative on top of the fp8 error)
    out_d = nc.dram_tensor("out", [S, D], BF, kind="ExternalOutput")
    # per-row 1/softmax-denominator, exported for the host rank-1 fixup
    rcp_d = nc.dram_tensor("rcpv", [S, 1], F32, kind="ExternalOutput")

    from contextlib import ExitStack

    with tile.TileContext(nc) as tc, ExitStack() as ctx:
        # ---- persistent SBUF arrays (live across both phases) ----
        pers = ctx.enter_context(tc.tile_pool(name="pers", bufs=1))
        xn_sb = pers.tile([P, ST, D], AT_DT)  # x normal: [t-part, t-tile, d]
        kt_sb = pers.tile([P, DT, S], SC_DT)  # K.T: [d-part, d-tile, s]
        # Q.T split per n-chunk so phase B's first scores don't wait on the
        # whole tensor's last tanh
        qt_n = [pers.tile([P, DT, NF], SC_DT, tag=f"qt{n}", name=f"qt{n}")
                for n in range(SN)]

        # scores PSUM pool opened before phase A so it gets banks disjoint
        # from the MLP pool - phase B's first matmul then has no released-pool
        # overlap dependency on phase A's tail
        psc = ctx.enter_context(tc.tile_pool(name="psum_sc", bufs=3, space="PSUM"))

        # ---- phase A: the four MLP layers ----
        with tc.tile_pool(name="phase_a", bufs=1) as pa, \
             tc.tile_pool(name="psum_mlp", bufs=4, space="PSUM") as pm:
            wp = pa  # weight tiles share the pool (fewer close barriers)
            # x.T split into per-n-chunk tiles so the first psum row's matmuls
            # only wait on the 1MB slice they read, not the whole array;
            # the n=0 chunk is additionally k-halved for an even earlier start
            KH = DT // 2
            xt_f = [pa.tile([P, KH, NF], MLP_DT, tag=f"xtf{h}", name=f"xtf{h}")
                    for h in range(2)]
            xt_n = [pa.tile([P, DT, NF], MLP_DT, tag=f"xt{n}", name=f"xt{n}")
                    for n in range(1, SN)]

            def xt_slice(n, k, w):
                if n == 0:
                    return xt_f[k // KH][:, k % KH:k % KH + w, :]
                return xt_n[n - 1][:, k:k + w, :]

            h1_sb = pa.tile([P, DT, S], MLP_DT)  # hidden activations (K then Q)
            # fp8 copy of hidden slices j=0,1 (the mixed DR pair's rhs);
            # layer 1's tanh writes these directly when MLP2_MIX
            h1_8 = pa.tile([P, 2, S], E4, name="h1_8") if MLP2_MIX else None

            # HAM warmup: throwaway matmuls while the first input DMAs are in
            # flight, so the real matmuls start at 2.4GHz
            warm_sb = pa.tile([P, NF], BF)
            nc.vector.memset(warm_sb, 0.0)
            warm_ps = pm.tile([P, NF], F32, tag="warm", bufs=1)
            NWARM = 13
            for i in range(NWARM):
                nc.tensor.matmul(warm_ps, warm_sb[:, 0:P], warm_sb,
                                 start=(i == 0), stop=(i == NWARM - 1))

            def mlp_layer(src, w_dram, dst, xdma=None, first=False,
                          w8_dram=None, dst8=None):
                # dst[j, s] = tanh(scale * sum_k w[k, j].T @ src[k, s])
                # one tile + one DMA per j-block so dep granularity is per-j.
                # w8_dram: fp8 weights for k-slices 0,1 -> one DR pair per
                # group (reads src's fp8 copy h1_8). dst8: fp8 destination
                # for output slices j=0,1 (feeds the next layer's DR pair).
                xt_r = xt_d.rearrange("(k p) s -> p k s", p=P)
                if first:
                    w_jf = [wp.tile([P, KH, P], MLP_DT, tag=f"wf{h}",
                                    name=f"wf{h}", bufs=2) for h in range(2)]
                    w_j = [wp.tile([P, DT, P], MLP_DT, tag=f"w{j}",
                                   name=f"w{j}", bufs=2) for j in range(1, DT)]
                    for h in range(2):
                        nc.sync.dma_start(out=xt_f[h], in_=xh_d[:, h, :, :])
                        nc.scalar.dma_start(
                            out=w_jf[h], in_=w_dram[:, 0, h * KH:(h + 1) * KH, :])
                    for j in range(1, DT):
                        nc.scalar.dma_start(out=w_j[j - 1], in_=w_dram[:, j, :, :])
                    for n in range(1, SN):
                        nc.sync.dma_start(
                            out=xt_n[n - 1], in_=xt_r[:, :, n * NF:(n + 1) * NF])

                    def lhs_sl(j, k, w):
                        return (w_jf[k // KH][:, k % KH:k % KH + w, :] if j == 0
                                else w_j[j - 1][:, k:k + w, :])
                else:
                    w_j = [wp.tile([P, DT, P], MLP_DT, tag=f"w{j}",
                                   name=f"w{j}", bufs=2) for j in range(DT)]
                    for j in range(DT):
                        nc.sync.dma_start(out=w_j[j], in_=w_dram[:, j, :, :])
                    if w8_dram is not None:
                        w8_sb = wp.tile([P, DT, 2, P], E4, tag="w8",
                                        name="w8", bufs=2)
                        nc.scalar.dma_start(out=w8_sb, in_=w8_dram[:, :, :, :])
                    if xdma is not None:
                        xdma()

                    def lhs_sl(j, k, w):
                        return w_j[j][:, k:k + w, :]

                def rhs_sl(n, k, w):
                    return (xt_slice(n, k, w) if src is None
                            else src[:, k:k + w, n * NF:(n + 1) * NF])

                loop = ([(j, n) for n in range(SN) for j in range(DT)] if first
                        else [(j, n) for j in range(DT) for n in range(SN)])
                for j, n in loop:
                    ps = pm.tile([P, NF], F32, tag="mlp")
                    if w8_dram is not None:
                        # k=0,1 as one fp8 DR pair, k=2..7 bf16 into the
                        # same accumulation group
                        nc.tensor.matmul(
                            ps, w8_sb[:, j, :, :],
                            h1_8[:, :, n * NF:(n + 1) * NF],
                            start=True, stop=False, perf_mode=DR)
                        for k in range(2, DT):
                            nc.tensor.matmul(ps, lhs_sl(j, k, 1),
                                             rhs_sl(n, k, 1),
                                             start=False, stop=(k == DT - 1))
                    else:
                        dr_matmuls(nc, ps,
                                   lambda k, w: lhs_sl(j, k, w),
                                   lambda k, w: rhs_sl(n, k, w),
                                   DT, MLP_FP8)
                    if dst8 is not None and j < 2:
                        dslice = dst8[:, j, n * NF:(n + 1) * NF]
                    else:
                        dslice = (dst[n][:, j, :] if isinstance(dst, list)
                                  else dst[:, j, n * NF:(n + 1) * NF])
                    nc.scalar.activation(out=dslice, in_=ps, func=AF.Tanh,
                                         scale=(1.0 / WS) if MLP_FP8 else 1.0)

            def load_xn():
                xn_r = xn_d.rearrange("(t p) d -> p t d", p=P)
                for t in range(0, ST, 4):
                    nc.sync.dma_start(out=xn_sb[:, t:t + 4, :],
                                      in_=xn_r[:, t:t + 4, :])

            if MLP2_MIX:
                mlp_layer(None, wk1_d, h1_sb, first=True, dst8=h1_8)
                mlp_layer(h1_sb, wk2_d, kt_sb, w8_dram=wk2p_d)
                mlp_layer(None, wq1_d, h1_sb, xdma=load_xn, dst8=h1_8)
                mlp_layer(h1_sb, wq2_d, qt_n, w8_dram=wq2p_d)
            else:
                mlp_layer(None, wk1_d, h1_sb, first=True)
                mlp_layer(h1_sb, wk2_d, kt_sb)
                mlp_layer(None, wq1_d, h1_sb, xdma=load_xn)
                mlp_layer(h1_sb, wq2_d, qt_n)

        # ---- phase B: transposed scores -> centered fp8 -> attended ----
        # scT[t, s] = Q[t].K[s] is computed with t on the psum partitions, so
        # exp lands DIRECTLY in the attended matmul's lhsT layout: no xbar
        # transpose, no cross-queue WAR coupling (the transpose chain was
        # pacing the PE in the row-major formulation). Softmax denominators
        # become partition-axis sums, recovered by a tiny ones-matmul per
        # s-block (~60 cycles/DR-pair on the PE).
        with tc.tile_pool(name="pb", bufs=1) as pb, \
             tc.tile_pool(name="psum_b", bufs=3, space="PSUM") as pbp:

            # Delta-W transposed, single-assignment: [t-part, t-block, s] fp8
            wq8 = pb.tile([P, ST, S], E4)
            ones8 = pb.tile([P, 2, 1], E4, tag="ones", name="ones8")
            nc.vector.memset(ones8, 1.0)

            # scores phase: exp(scT)-WC in fp8, s-chunk-major so the s-chunks
            # the first attended blocks need are finished long before the
            # phase boundary (no drain bubble)
            for n in range(SN):
                for tb in range(ST):
                    nq, qo = tb // SN, (tb % SN) * P
                    ps = psc.tile([P, NF], F32, tag="sc")
                    dr_matmuls(
                        nc, ps,
                        lambda k, w: qt_n[nq][:, k:k + w, qo:qo + P],
                        lambda k, w: kt_sb[:, k:k + w, n * NF:(n + 1) * NF],
                        DT, SC_FP8)
                    # scores are bounded (|sc/32| < ~3): exp without max-shift
                    est = pb.tile([P, NF], BF, tag="est", bufs=4)
                    nc.scalar.activation(out=est, in_=ps, func=AF.Exp,
                                         scale=float(SCALE))
                    nc.vector.tensor_scalar(
                        wq8[:, tb, n * NF:(n + 1) * NF], est, float(WC), None,
                        mybir.AluOpType.subtract)

            def attended(i):
                # softmax denominators for s-block i: rowsum of Delta-W via
                # ones-matmul (partition-axis sum), then 1/(rs + S*WC)
                rs = pbp.tile([P, 1], F32, tag="rs", bufs=1)
                for t2 in range(ST // 2):
                    nc.tensor.matmul(
                        rs, wq8[:, 2 * t2:2 * t2 + 2, i * P:(i + 1) * P],
                        ones8, start=(t2 == 0), stop=(t2 == ST // 2 - 1),
                        perf_mode=DR)
                rcp = pb.tile([P, 1], F32, tag="rcp", bufs=4)
                nc.vector.tensor_scalar(rcp, rs, float(S) * float(WC), None,
                                        mybir.AluOpType.add)
                nc.vector.reciprocal(rcp, rcp)
                nc.sync.dma_start(out=rcp_d[i * P:(i + 1) * P, :], in_=rcp)
                outst = pb.tile([P, D], BF, tag="outst", bufs=3)
                for n in range(DN):
                    # 4 bufs: with 3, group g's start waits on mul(g-3) and
                    # the scalar queue's latency leaks into the PE
                    ps = pbp.tile([P, NF], F32, tag="at", bufs=4)
                    dr_matmuls(
                        nc, ps,
                        lambda t, w: wq8[:, t:t + w, i * P:(i + 1) * P],
                        lambda t, w: xn_sb[:, t:t + w, n * NF:(n + 1) * NF],
                        ST, AT_FP8)
                    # fold the softmax normalization into the PSUM->SBUF copy
                    nc.scalar.mul(outst[:, n * NF:(n + 1) * NF], ps, rcp)
                    nc.sync.dma_start(
                        out=out_d[i * P:(i + 1) * P, n * NF:(n + 1) * NF],
                        in_=outst[:, n * NF:(n + 1) * NF],
                    )

            for i in range(ST):
                attended(i)

    nc.compile()
    return nc


_NC = None


def _get_nc():
    global _NC
    if _NC is None:
        _NC = build_nc()
    return _NC


NP_MLP = ml_dtypes.float8_e4m3 if MLP_FP8 else ml_dtypes.bfloat16
NP_SC = ml_dtypes.float8_e4m3 if SC_FP8 else ml_dtypes.bfloat16
NP_AT = ml_dtypes.float8_e4m3 if AT_FP8 else ml_dtypes.bfloat16


def _prep_w(w):
    """[d_out, d_in] f32 -> [p, j, k, c] of (WS*w.T) (k,p index d_in; j,c d_out)."""
    wt = np.asarray(w, dtype=np.float32).T
    if MLP_FP8:
        wt = wt * np.float32(WS)
    wt = wt.reshape(DT, P, DT, P).transpose(1, 2, 0, 3)
    return np.ascontiguousarray(wt).astype(NP_MLP)


def _prep_w8(w):
    """k-slices 0,1 of w.T as [p, j, 2, c] e4m3, unscaled."""
    wt = np.asarray(w, dtype=np.float32).T
    wt = wt.reshape(DT, P, DT, P).transpose(1, 2, 0, 3)[:, :, 0:2, :]
    return np.ascontiguousarray(wt).astype(ml_dtypes.float8_e4m3)


def make_in_maps(sequence, Kw1, Kw2, Qw1, Qw2):
    seq = np.ascontiguousarray(np.transpose(np.asarray(sequence), (1, 0, 2)))  # [B, S, D]
    ws = {"wk1": _prep_w(Kw1), "wk2": _prep_w(Kw2),
          "wq1": _prep_w(Qw1), "wq2": _prep_w(Qw2)}
    if MLP2_MIX:
        ws["wk2p"] = _prep_w8(Kw2)
        ws["wq2p"] = _prep_w8(Qw2)
    in_maps = []
    colsums = []
    for b in range(B):
        xb = seq[b]
        xt = np.ascontiguousarray(xb.T).astype(NP_MLP)
        # [P, 2, KH, NF]: xh[p, h, q, s] = xt[(h*KH + q)*P + p, s] for s < NF
        xh = np.ascontiguousarray(
            xt[:, 0:NF].reshape(2, DT // 2, P, NF).transpose(2, 0, 1, 3))
        m = {"xn": xb.astype(NP_AT), "xt": xt, "xh": xh}
        m.update(ws)
        in_maps.append(m)
        colsums.append(xb.astype(np.float32).sum(axis=0))  # [D]
    return in_maps, colsums


def kernel(sequence, Kw1, Kw2, Qw1, Qw2):
    nc = _get_nc()
    in_maps, colsums = make_in_maps(sequence, Kw1, Kw2, Qw1, Qw2)
    res = run_bass_kernel_spmd(nc, in_maps, core_ids=list(range(B)))
    outs = []
    for b in range(B):
        ob = np.asarray(res.results[b]["out"], dtype=np.float32)
        if AT_FP8:
            # add back the rank-1 mean term removed by the W centering
            rcpv = np.asarray(res.results[b]["rcpv"], dtype=np.float32)[:, 0]
            ob = ob + np.float32(WC) * np.outer(rcpv, colsums[b])
        outs.append(ob)
    return np.stack(outs, axis=1).astype(np.float32)
